# revision 2
# baseline (speedup 1.0000x reference)
"""AEGRU Trainium kernel builder V3 (see V2 notes).

Changes vs V1:
- Encoder: input-side matmuls (gi) batched over blocks of T steps with
  biases folded in; recurrent loop only does the 48 gh pairs per layer-step.
  Layer0 of block k interleaved with layer1 of block k-1 so PE and DVE/ACT
  of the two recurrences overlap.
- Vec chain: h' = zc*(n-h) + h with zc = sigmoid(-gz) on ACT (off critical
  path), saving ops and shortening the serial chain.
- dtype of weights parameterizable (bf16 / fp8e4) for LDWEIGHTS speed.
"""
import sys

sys.path.insert(0, "/opt/trn_rl_repo")

from contextlib import ExitStack

import ml_dtypes
import numpy as np

import concourse.bass as bass
import concourse.mybir as mybir
import concourse.tile as tile
from concourse import bacc
from concourse.bass import ds, ts

F32 = mybir.dt.float32
BF16 = mybir.dt.bfloat16
FP8 = mybir.dt.float8e4
AF = mybir.ActivationFunctionType

B, D, H = 128, 128, 512
NCORES = 8
BL = 16
G3 = 1536
NKC = 4
bf16 = ml_dtypes.bfloat16
f8e4 = ml_dtypes.float8_e4m3


def np_wdt(wdt):
    return bf16 if wdt == BF16 else f8e4


def _wT_chunks(W):
    out_d, in_d = W.shape
    nk = in_d // 128
    Wt = W.T.reshape(nk, 128, out_d)
    return np.ascontiguousarray(Wt.transpose(1, 0, 2).reshape(128, nk * out_d))


def pack_inputs(inputs, S, wdt=BF16):
    x = np.asarray(inputs["x"], np.float32)
    ndt = np_wdt(wdt)
    shared = {}
    for pre, tag in [("enc_", "e"), ("dec_", "d")]:
        for L in (0, 1):
            shared[f"wih{tag}{L}"] = _wT_chunks(
                np.asarray(inputs[f"{pre}Wih{L}"], np.float32)
            ).astype(ndt)
            shared[f"whh{tag}{L}"] = _wT_chunks(
                np.asarray(inputs[f"{pre}Whh{L}"], np.float32)
            ).astype(ndt)
            bih = np.asarray(inputs[f"{pre}bih{L}"], np.float32)
            bhh = np.asarray(inputs[f"{pre}bhh{L}"], np.float32)
            # per-gate-tile [128, 12] scalars: rz tiles get bih+bhh, n tiles bih
            bcol = np.zeros((128, 12), np.float32)
            for g in range(8):
                bcol[:, g] = (bih + bhh)[g * 128 : (g + 1) * 128]
            for j in range(4):
                bcol[:, 8 + j] = bih[1024 + j * 128 : 1024 + (j + 1) * 128]
            shared[f"bcol{tag}{L}"] = bcol
            # K=16 bias matmul operands: psum[p, 16g+b] init = bmat[g, p]
            bmat = np.zeros((16, 128), np.float32)
            for g in range(8):
                bmat[g] = (bih + bhh)[g * 128 : (g + 1) * 128]
            for j in range(4):
                bmat[8 + j] = bhh[1024 + j * 128 : 1024 + (j + 1) * 128]
                bmat[12 + j] = bih[1024 + j * 128 : 1024 + (j + 1) * 128]
            shared[f"bmat{tag}{L}"] = bmat.astype(bf16)
            # bias rows for K=1 matmul accumulation (decoder): [1, 16*128]
            # psum slice order: 8 rz tiles (bih+bhh), 4 ghn tiles (bhh_n),
            # 4 gin tiles (bih_n)
            brow = np.zeros(16 * 128, np.float32)
            brow[:1024] = (bih + bhh)[:1024]
            brow[1024:1536] = bhh[1024:]
            brow[1536:] = bih[1024:]
            shared[f"brow{tag}{L}"] = brow.reshape(1, 16 * 128).astype(bf16)
            bias_full = np.zeros((128, 256), np.float32)
            rz = (bih + bhh)[:1024].reshape(8, 128)
            hn_ = bhh[1024:].reshape(4, 128)
            in_ = bih[1024:].reshape(4, 128)
            for g in range(8):
                bias_full[:, g * 16 : (g + 1) * 16] = rz[g][:, None]
            for j in range(4):
                bias_full[:, 128 + j * 16 : 128 + (j + 1) * 16] = hn_[j][:, None]
                bias_full[:, 192 + j * 16 : 192 + (j + 1) * 16] = in_[j][:, None]
            shared[f"bias{tag}{L}"] = bias_full
            # [128, 64] bhh_n broadcast (encoder in-loop ghn add)
            bhn = np.zeros((128, 64), np.float32)
            hn = bhh[1024:].reshape(4, 128)
            for j in range(4):
                bhn[:, j * 16 : (j + 1) * 16] = hn[j][:, None]
            shared[f"bhn{tag}{L}"] = bhn
    shared["wout"] = _wT_chunks(np.asarray(inputs["Wout"], np.float32)).astype(ndt)
    shared["bout"] = np.asarray(inputs["bout"], np.float32).reshape(128, 1)
    bsel = np.zeros((16, 256), np.float32)
    for g in range(16):
        bsel[g, g * 16 : (g + 1) * 16] = 1.0
    shared["bsel"] = bsel.astype(bf16)

    in_maps = []
    for i in range(NCORES):
        xs = x[i * BL : (i + 1) * BL, :S]
        xt = np.ascontiguousarray(xs.transpose(1, 2, 0)).astype(bf16)  # [S, D, 16]
        m = {"x": xt}
        m.update(shared)
        in_maps.append(m)
    return in_maps


def unpack_outputs(results, S):
    recon = np.empty((B, S, D), np.float32)
    z = np.empty((B, H), np.float32)
    for i, r in enumerate(results):
        recon[i * BL : (i + 1) * BL] = r["recon"].transpose(2, 0, 1)
        zc = r["zout"].reshape(128, 4, 16)
        z[i * BL : (i + 1) * BL] = zc.transpose(2, 1, 0).reshape(BL, H)
    return recon, z


def build(S, T=32, wdt=BF16):
    assert S % T == 0
    NBLK = S // T
    NT = T * BL
    nc = bacc.Bacc("TRN2", target_bir_lowering=False, debug=False)

    x_d = nc.dram_tensor("x", [S, 128, BL], BF16, kind="ExternalInput").ap()
    wd = {}
    for tag in ("e", "d"):
        for L in (0, 1):
            nin = 1 if L == 0 else NKC
            for nm, shape, dt in [
                (f"wih{tag}{L}", [128, nin * G3], wdt),
                (f"whh{tag}{L}", [128, NKC * G3], wdt),
                (f"bcol{tag}{L}", [128, 12], F32),
                (f"bmat{tag}{L}", [16, 128], BF16),
                (f"bias{tag}{L}", [128, 256], F32),
                (f"bhn{tag}{L}", [128, 64], F32),
            ]:
                wd[nm] = nc.dram_tensor(nm, shape, dt, kind="ExternalInput").ap()
    wd["wout"] = nc.dram_tensor("wout", [128, NKC * 128], wdt, kind="ExternalInput").ap()
    wd["bout"] = nc.dram_tensor("bout", [128, 1], F32, kind="ExternalInput").ap()
    wd["bsel"] = nc.dram_tensor("bsel", [16, 256], BF16, kind="ExternalInput").ap()

    recon_d = nc.dram_tensor("recon", [S, 128, BL], F32, kind="ExternalOutput").ap()
    zout_d = nc.dram_tensor("zout", [128, NKC * BL], F32, kind="ExternalOutput").ap()

    with tile.TileContext(nc) as tc, ExitStack() as ctx:
        wpool = ctx.enter_context(tc.tile_pool(name="weights", bufs=1))
        hpool = ctx.enter_context(tc.tile_pool(name="state", bufs=3))
        work = ctx.enter_context(tc.tile_pool(name="work", bufs=3))
        xin = ctx.enter_context(tc.tile_pool(name="xin", bufs=2))
        gip = ctx.enter_context(tc.tile_pool(name="gip", bufs=2))
        opool = ctx.enter_context(tc.tile_pool(name="outp", bufs=4))
        psumA = ctx.enter_context(tc.tile_pool(name="psumA", bufs=2, space="PSUM"))
        psumB = ctx.enter_context(tc.tile_pool(name="psumB", bufs=2, space="PSUM"))
        psumg = ctx.enter_context(tc.tile_pool(name="psumg", bufs=4, space="PSUM"))

        w = {}
        for k, ap in wd.items():
            t = wpool.tile(list(ap.shape), ap.dtype, tag=k, name=k)
            nc.sync.dma_start(out=t[:], in_=ap[:])
            w[k] = t

        def chunk(h, c):
            if len(h.shape) == 3:
                return h[:, c, :]
            return h[:, ts(c, BL)]

        def batched_gi(wih, nin, rhs_blk, bcol, tagp):
            """gi for T steps: [128, 12, NT] bf16, biases folded in."""
            gi = gip.tile([128, 12, NT], BF16, tag=f"gi{tagp}", name=f"gi{tagp}")
            for g in range(12):
                pg = psumg.tile([128, NT], F32, tag="pgi", name="pgi")
                for c in range(nin):
                    nc.tensor.matmul(
                        pg[:],
                        wih[:, ds(c * G3 + g * 128, 128)],
                        chunk(rhs_blk, c),
                        start=(c == 0),
                        stop=(c == nin - 1),
                    )
                nc.vector.tensor_scalar_add(gi[:, g, :], pg[:], bcol[:, g : g + 1])
            return gi

        def vec_tail(ps_or_t2_src, r, zc, h_rd, gi_n, tagp, out_slice=None):
            """t1..h' chain shared by enc/dec. gi_n: [128,(4,)64]-view or None
            (dec: gi_n lives in psum cols 192:256 -> pass ps slice)."""
            pass  # placeholder (unused)

        def enc_step(whh, h_rd, gi, tl, bhn, tagp, hf, hb_out_slice=None):
            """Recurrent step. h_rd: bf16 view for matmul rhs; hf: f32 master
            used by the elementwise path. Returns (new_hb_or_None, new_hf)."""
            pool_ = psumA if tagp == "0" else psumB
            ps = pool_.tile([128, 192], F32, tag=f"ps{tagp}", name=f"ps{tagp}", padded_shape=[128, 512])
            for g in range(12):
                for c in range(NKC):
                    nc.tensor.matmul(
                        ps[:, ts(g, BL)],
                        whh[:, ds(c * G3 + g * 128, 128)],
                        chunk(h_rd, c),
                        start=(c == 0),
                        stop=(c == NKC - 1),
                    )
            grz = work.tile([128, 128], F32, tag=f"grz{tagp}", name=f"grz{tagp}")
            nc.vector.tensor_add(grz[:, 0:64], ps[:, 0:64], gi[:, 0:4, ts(tl, BL)])
            nc.vector.tensor_add(grz[:, 64:128], ps[:, 64:128], gi[:, 4:8, ts(tl, BL)])
            r = work.tile([128, 64], F32, tag=f"r{tagp}", name=f"r{tagp}")
            nc.scalar.activation(r[:], grz[:, 0:64], AF.Sigmoid)
            zc = work.tile([128, 64], F32, tag=f"zc{tagp}", name=f"zc{tagp}")
            nc.scalar.activation(zc[:], grz[:, 64:128], AF.Sigmoid, scale=-1.0)
            ghn = work.tile([128, 64], F32, tag=f"ghn{tagp}", name=f"ghn{tagp}")
            nc.vector.tensor_add(ghn[:], ps[:, 128:192], bhn[:])
            t1 = work.tile([128, 64], F32, tag=f"t1{tagp}", name=f"t1{tagp}")
            nc.vector.tensor_mul(t1[:], r[:], ghn[:])
            t2 = work.tile([128, 64], F32, tag=f"t2{tagp}", name=f"t2{tagp}")
            nc.vector.tensor_add(t2[:], t1[:], gi[:, 8:12, ts(tl, BL)])
            nt = work.tile([128, 64], F32, tag=f"nt{tagp}", name=f"nt{tagp}")
            nc.scalar.activation(nt[:], t2[:], AF.Tanh)
            ch = work.tile([128, 64], F32, tag=f"ch{tagp}", name=f"ch{tagp}")
            nc.vector.tensor_mul(ch[:], zc[:], hf[:])
            hc = work.tile([128, 64], F32, tag=f"hc{tagp}", name=f"hc{tagp}")
            nc.vector.tensor_sub(hc[:], hf[:], ch[:])
            zn = work.tile([128, 64], F32, tag=f"zn{tagp}", name=f"zn{tagp}")
            nc.vector.tensor_mul(zn[:], zc[:], nt[:])
            hf2 = hpool.tile([128, 64], F32, tag=f"hf{tagp}", name=f"hf{tagp}")
            if hb_out_slice is not None:
                nc.vector.tensor_add(hb_out_slice, zn[:], hc[:])
                nc.vector.tensor_add(hf2[:], zn[:], hc[:])
                return None, hf2
            hb2 = hpool.tile([128, 64], BF16, tag=f"hb2{tagp}", name=f"hb2{tagp}")
            nc.vector.tensor_add(hb2[:], zn[:], hc[:])
            nc.vector.tensor_add(hf2[:], zn[:], hc[:])
            return hb2, hf2

        # ---------------- encoder ----------------
        prev_gi1 = None
        h0z = hpool.tile([128, 64], BF16, tag="h0z", name="h0z")
        nc.vector.memset(h0z[:], 0.0)
        h1b = hpool.tile([128, 64], BF16, tag="h1b", name="h1b")
        nc.vector.memset(h1b[:], 0.0)
        h0_rd = h0z
        h0fm = hpool.tile([128, 64], F32, tag="hf0", name="hf0init")
        nc.vector.memset(h0fm[:], 0.0)
        h1fm = hpool.tile([128, 64], F32, tag="hf1", name="hf1init")
        nc.vector.memset(h1fm[:], 0.0)
        h0blk = None
        for k in range(NBLK):
            xblk = xin.tile([128, 1, NT], BF16, tag="xblk", name="xblk")
            nc.sync.dma_start(
                out=xblk[:, 0, :], in_=x_d[ds(k * T, T)].rearrange("t p b -> p t b")
            )
            gi0 = batched_gi(w["wihe0"], 1, xblk, w["bcole0"], "0")
            h0blk = gip.tile([128, NKC, NT], BF16, tag="h0blk", name="h0blk")
            for tl in range(T):
                _, h0fm = enc_step(
                    w["whhe0"], h0_rd, gi0, tl, w["bhne0"], "0", h0fm,
                    hb_out_slice=h0blk[:, :, ts(tl, BL)],
                )
                h0_rd = h0blk[:, :, ts(tl, BL)]
                if prev_gi1 is not None:
                    h1b, h1fm = enc_step(
                        w["whhe1"], h1b, prev_gi1, tl, w["bhne1"], "1", h1fm
                    )
            prev_gi1 = batched_gi(w["wihe1"], NKC, h0blk, w["bcole1"], "1")
        for tl in range(T):
            h1b, h1fm = enc_step(w["whhe1"], h1b, prev_gi1, tl, w["bhne1"], "1", h1fm)

        nc.sync.dma_start(out=zout_d[:], in_=h1fm[:])

        h0db = hpool.tile([128, 64], BF16, tag="h0db", name="h0db")
        nc.vector.tensor_copy(h0db[:], h0blk[:, :, ts(T - 1, BL)])
        h0dfm = h0fm
        h1dfm = h1fm

        # ---------------- decoder ----------------
        def dec_bias_gh(ps, bmat, whh, h_rd):
            """bank-open bias init (K=16 rank-16 matmul) + all recurrent
            chunks, all accumulating."""
            nc.tensor.matmul(
                ps[:], bmat[:], w["bsel"][:], start=True, stop=False,
                skip_group_check=True,
            )
            for g in range(12):
                for c in range(NKC):
                    nc.tensor.matmul(
                        ps[:, ts(g, BL)],
                        whh[:, ds(c * G3 + g * 128, 128)],
                        chunk(h_rd, c),
                        start=False,
                        stop=(c == NKC - 1) and g >= 8,
                        skip_group_check=True,
                    )

        def dec_gi(ps, wih, nin, gi_rhs):
            for g in range(8):
                for c in range(nin):
                    nc.tensor.matmul(
                        ps[:, ts(g, BL)],
                        wih[:, ds(c * G3 + g * 128, 128)],
                        chunk(gi_rhs, c) if nin > 1 else gi_rhs[:],
                        start=False,
                        stop=(c == nin - 1),
                        skip_group_check=True,
                    )
            for j in range(4):
                for c in range(nin):
                    nc.tensor.matmul(
                        ps[:, ts(12 + j, BL)],
                        wih[:, ds(c * G3 + (8 + j) * 128, 128)],
                        chunk(gi_rhs, c) if nin > 1 else gi_rhs[:],
                        start=False,
                        stop=(c == nin - 1),
                        skip_group_check=True,
                    )

        def dec_vec(ps, bias_full, hf, tagp):
            r = work.tile([128, 64], F32, tag=f"dr{tagp}", name=f"dr{tagp}")
            nc.scalar.activation(r[:], ps[:, 0:64], AF.Sigmoid)
            zc = work.tile([128, 64], F32, tag=f"dzc{tagp}", name=f"dzc{tagp}")
            nc.scalar.activation(zc[:], ps[:, 64:128], AF.Sigmoid, scale=-1.0)
            t1 = work.tile([128, 64], F32, tag=f"dt1{tagp}", name=f"dt1{tagp}")
            nc.vector.tensor_mul(t1[:], r[:], ps[:, 128:192])
            t2 = work.tile([128, 64], F32, tag=f"dt2{tagp}", name=f"dt2{tagp}")
            nc.vector.tensor_add(t2[:], t1[:], ps[:, 192:256])
            nt = work.tile([128, 64], F32, tag=f"dnt{tagp}", name=f"dnt{tagp}")
            nc.scalar.activation(nt[:], t2[:], AF.Tanh)
            ch = work.tile([128, 64], F32, tag=f"dch{tagp}", name=f"dch{tagp}")
            nc.vector.tensor_mul(ch[:], zc[:], hf[:])
            hc = work.tile([128, 64], F32, tag=f"dhc{tagp}", name=f"dhc{tagp}")
            nc.vector.tensor_sub(hc[:], hf[:], ch[:])
            zn = work.tile([128, 64], F32, tag=f"dzn{tagp}", name=f"dzn{tagp}")
            nc.vector.tensor_mul(zn[:], zc[:], nt[:])
            hb2 = hpool.tile([128, 64], BF16, tag=f"dhb{tagp}", name=f"dhb{tagp}")
            nc.vector.tensor_add(hb2[:], zn[:], hc[:])
            hf2 = hpool.tile([128, 64], F32, tag=f"dhf{tagp}", name=f"dhf{tagp}")
            nc.vector.tensor_add(hf2[:], zn[:], hc[:])
            return hb2, hf2

        predb = hpool.tile([128, BL], BF16, tag="predb", name="predb")
        nc.vector.memset(predb[:], 0.0)
        h1d = h1b
        for t in range(S):
            ps0 = psumA.tile([128, 256], F32, tag="ps0", name="ps0", padded_shape=[128, 512])
            dec_bias_gh(ps0, w["bmatd0"], w["whhd0"], h0db)
            if t > 0:
                # pred(t-1) = Wout @ h1(t-1); issued after gh0 so vec1(t-1)
                # has finished producing h1d by the time PE reaches it
                psp = psumg.tile([128, BL], F32, tag="pgi", name="psp", padded_shape=[128, 512])
                for c in range(NKC):
                    nc.tensor.matmul(
                        psp[:],
                        w["wout"][:, ts(c, 128)],
                        h1d[:, ts(c, BL)],
                        start=(c == 0),
                        stop=(c == NKC - 1),
                    )
                predf = opool.tile([128, BL], F32, tag="predf", name="predf")
                nc.vector.tensor_scalar_add(predf[:], psp[:], w["bout"][:])
                predb = hpool.tile([128, BL], BF16, tag="predb", name="predb")
                nc.vector.tensor_scalar_add(predb[:], psp[:], w["bout"][:])
                nc.sync.dma_start(out=recon_d[t - 1], in_=predf[:])
            dec_gi(ps0, w["wihd0"], 1, predb)
            ps1 = psumB.tile([128, 256], F32, tag="ps1", name="ps1", padded_shape=[128, 512])
            dec_bias_gh(ps1, w["bmatd1"], w["whhd1"], h1d)
            h0db, h0dfm = dec_vec(ps0, w["biasd0"], h0dfm, "0")
            dec_gi(ps1, w["wihd1"], NKC, h0db)
            h1d, h1dfm = dec_vec(ps1, w["biasd1"], h1dfm, "1")
        # final step's projection
        psp = psumg.tile([128, BL], F32, tag="pgi", name="psp", padded_shape=[128, 512])
        for c in range(NKC):
            nc.tensor.matmul(
                psp[:],
                w["wout"][:, ts(c, 128)],
                h1d[:, ts(c, BL)],
                start=(c == 0),
                stop=(c == NKC - 1),
            )
        predf = opool.tile([128, BL], F32, tag="predf", name="predf")
        nc.vector.tensor_scalar_add(predf[:], psp[:], w["bout"][:])
        nc.sync.dma_start(out=recon_d[S - 1], in_=predf[:])

    nc.compile()
    return nc


# ============================================================================
# Self-contained harness entrypoint
# ============================================================================
_CACHE = {}


def _get_nc(S):
    if S not in _CACHE:
        _CACHE[S] = build(S)
    return _CACHE[S]


class _Runner:
    """Caches the jitted 8-core PJRT executable so repeated kernel() calls
    skip re-tracing/compiling (mirrors bass2jax.run_bass_via_pjrt)."""

    def __init__(self, nc, n_cores=NCORES):
        import jax
        from jax.sharding import Mesh, PartitionSpec
        from jax.experimental.shard_map import shard_map
        from concourse import bass2jax

        bass2jax.install_neuronx_cc_hook()
        self.jax = jax
        in_names, out_names, out_avals = [], [], []
        pname = nc.partition_id_tensor.name if nc.partition_id_tensor else None
        for alloc in nc.m.functions[0].allocations:
            if not isinstance(alloc, mybir.MemoryLocationSet):
                continue
            name = alloc.memorylocations[0].name
            if alloc.kind == "ExternalInput":
                if name != pname:
                    in_names.append(name)
            elif alloc.kind == "ExternalOutput":
                out_names.append(name)
                out_avals.append(
                    jax.core.ShapedArray(
                        tuple(alloc.tensor_shape), mybir.dt.np(alloc.dtype)
                    )
                )
        self.in_names, self.out_names, self.out_avals = in_names, out_names, out_avals
        n_params, n_outs = len(in_names), len(out_avals)
        all_in = list(in_names) + list(out_names) + ([pname] if pname else [])
        donate = tuple(range(n_params, n_params + n_outs))

        def _body(*args):
            operands = list(args)
            if pname is not None:
                operands.append(bass2jax.partition_id_tensor())
            return tuple(
                bass2jax._bass_exec_p.bind(
                    *operands,
                    out_avals=tuple(out_avals),
                    in_names=tuple(all_in),
                    out_names=tuple(out_names),
                    lowering_input_output_aliases=(),
                    sim_require_finite=True,
                    sim_require_nnan=True,
                    nc=nc,
                )
            )

        devices = jax.devices()[:n_cores]
        mesh = Mesh(np.asarray(devices), ("core",))
        self.sharding = jax.sharding.NamedSharding(mesh, PartitionSpec("core"))
        self.fn = jax.jit(
            shard_map(
                _body,
                mesh=mesh,
                in_specs=(PartitionSpec("core"),) * (n_params + n_outs),
                out_specs=(PartitionSpec("core"),) * n_outs,
                check_rep=False,
            ),
            donate_argnums=donate,
            keep_unused=True,
        )
        self.n_cores = n_cores

    def run(self, in_maps):
        jax = self.jax
        concat_in = [
            np.concatenate([np.asarray(m[n]) for m in in_maps], axis=0)
            for n in self.in_names
        ]
        dev_in = [jax.device_put(a, self.sharding) for a in concat_in]
        zeros = [
            jax.device_put(
                np.zeros((self.n_cores * a.shape[0], *a.shape[1:]), a.dtype),
                self.sharding,
            )
            for a in self.out_avals
        ]
        outs = self.fn(*dev_in, *zeros)
        return [
            {
                n: np.asarray(outs[i]).reshape(self.n_cores, *self.out_avals[i].shape)[c]
                for i, n in enumerate(self.out_names)
            }
            for c in range(self.n_cores)
        ]


_RUNNERS = {}


def kernel(**inputs):
    """Full-input AEGRU autoencoder on 8 TRN2 NeuronCores (data-parallel
    over batch). Returns (reconstructed [B,S,D] f32, z [B,H] f32).
    """
    S = int(np.asarray(inputs["x"]).shape[1])
    nc = _get_nc(S)
    in_maps = pack_inputs(inputs, S)
    results = None
    for _ in range(2):  # cached-executable path, retry once on device hiccup
        try:
            if S not in _RUNNERS:
                _RUNNERS[S] = _Runner(nc)
            results = _RUNNERS[S].run(in_maps)
            break
        except Exception:  # noqa: BLE001
            _RUNNERS.pop(S, None)
    if results is None:  # robust fallback
        from concourse.bass_utils import run_bass_kernel_spmd

        res = run_bass_kernel_spmd(nc, in_maps, core_ids=list(range(NCORES)))
        results = res.results
    recon, z = unpack_outputs(results, S)
    return recon, z
